# revision 9
# baseline (speedup 1.0000x reference)
"""BitConv1d Trainium2 kernel.

Computes out[n,o,l] = conv1d(x, sign(w), pad=1) * mean(|w|) * scale, which is
mathematically identical to the reference

    x_scale = clip(mean(|x|, axis=(1,2)), 1e-5)
    out = conv1d(x / x_scale, sign(w), pad=1) * mean(|w|) * x_scale * scale

because conv is linear in x so the per-sample x_scale cancels exactly.

Sharding: data-parallel over batch N=16 across 8 cores (2 samples/core).

Device math: the PE array's native datapath is FP22 (e10m11).  float32
matmuls cost 4 passes; float32r costs 1 pass but rounds operands to
FP22.  Since sign(w) ∈ {-1,0,1} is FP22-exact, we split
    hi = round_fp22(x)    (DVE f32 -> f32r convert on write)
    lo = x - hi           (<= 12 significant bits -> FP22-exact)
and accumulate two float32r matmul passes into fp32 PSUM.  Both operands
are FP22-exact in SBUF, so every PE product is exact and the result
matches a true fp32 conv up to fp32 accumulation order, at 2x the
bf16-rate cost instead of 4x.
"""

import numpy as np

# Problem geometry (hardcoded per contract).
N, C, L, KW = 16, 512, 4096, 3
NCORES = 8
NS = N // NCORES          # samples per core
P = 128                   # partitions
PC = C // P               # input-channel chunks
OC = C // P               # output-channel chunks
NTILE = 512               # moving free-dim per matmul

_CACHE = {}


def _build_nc(ns=NS, c=C, length=L, kw=KW):
    from contextlib import ExitStack
    from concourse import bacc, tile, mybir

    f32 = mybir.dt.float32
    f32r = mybir.dt.float32r
    Alu = mybir.AluOpType
    Act = mybir.ActivationFunctionType

    pc_n = c // P
    oc_n = c // P
    lh_n = 2                  # L halves (psum double buffering)
    hw = length // lh_n       # columns per half
    lt_n = hw // NTILE        # matmul tiles per half
    wcols = hw + 2            # with 1-col halo on each side

    nc = bacc.Bacc("TRN2", target_bir_lowering=False, debug=False)

    x_d = nc.dram_tensor("x", [ns, c, length], f32, kind="ExternalInput")
    w_d = nc.dram_tensor("wt", [kw, c, c], f32, kind="ExternalInput")
    s_d = nc.dram_tensor("scale", [1, 1], f32, kind="ExternalInput")
    o_d = nc.dram_tensor("out", [ns, c, length], f32, kind="ExternalOutput")

    with tile.TileContext(nc) as tc, ExitStack() as ctx:
        consts = ctx.enter_context(tc.tile_pool(name="consts", bufs=1))
        wst_p = ctx.enter_context(tc.tile_pool(name="wst", bufs=2))
        wsgn_p = ctx.enter_context(tc.tile_pool(name="wsgn", bufs=kw * pc_n))
        xs_p = ctx.enter_context(tc.tile_pool(name="xs", bufs=2))
        hi_p = ctx.enter_context(tc.tile_pool(name="hi", bufs=2 * pc_n))
        lo_p = ctx.enter_context(tc.tile_pool(name="lo", bufs=2 * pc_n))
        out_p = ctx.enter_context(tc.tile_pool(name="outs", bufs=2))
        psum_p = ctx.enter_context(
            tc.tile_pool(name="psum", bufs=2, space="PSUM")
        )

        # ---------- setup: scale, sign(w), w_scale ----------
        sc = consts.tile([1, 1], f32, tag="sc")
        nc.sync.dma_start(sc[:, :], s_d[:, :])

        ones_col = consts.tile([P, 1], f32, tag="ones_col")
        nc.gpsimd.memset(ones_col[:], 1.0)
        ones_row = consts.tile([1, P], f32, tag="ones_row")
        nc.gpsimd.memset(ones_row[:], 1.0)

        partials = consts.tile([P, kw * pc_n], f32, tag="partials")
        wsgn = []
        for k in range(kw):
            for pc in range(pc_n):
                wst = wst_p.tile([P, c], f32, tag="wst")
                nc.sync.dma_start(wst[:], w_d[k, pc * P:(pc + 1) * P, :])
                j = k * pc_n + pc
                nc.vector.tensor_reduce(
                    partials[:, j:j + 1], wst[:], mybir.AxisListType.X,
                    Alu.add, apply_absolute_value=True,
                )
                wt = wsgn_p.tile([P, c], f32r, tag="wsgn")
                nc.scalar.sign(wt[:], wst[:])
                wsgn.append(wt)

        part1 = consts.tile([P, 1], f32, tag="part1")
        nc.vector.tensor_reduce(
            part1[:], partials[:], mybir.AxisListType.X, Alu.add
        )
        ps0 = psum_p.tile([P, hw], f32, tag="psum")
        nc.tensor.matmul(ps0[0:1, 0:1], part1[:], ones_col[:],
                         start=True, stop=True)
        tot = consts.tile([1, 1], f32, tag="tot")
        nc.vector.tensor_copy(tot[:], ps0[0:1, 0:1])
        c1 = consts.tile([1, 1], f32, tag="c1")
        nc.vector.tensor_tensor(c1[:], tot[:], sc[:], op=Alu.mult)
        nc.vector.tensor_scalar_mul(c1[:], c1[:], 1.0 / (c * c * kw))
        ps1 = psum_p.tile([P, hw], f32, tag="psum")
        nc.tensor.matmul(ps1[:, 0:1], ones_row[:], c1[:],
                         start=True, stop=True)
        cb = consts.tile([P, 1], f32, tag="cb")
        nc.vector.tensor_copy(cb[:], ps1[:, 0:1])

        # ---------- main loop ----------
        n_acc = 2 * pc_n * kw
        for s in range(ns):
            for h in range(lh_n):
                his, los = [], []
                for pc in range(pc_n):
                    xs = xs_p.tile([P, wcols], f32, tag="xs")
                    rows = slice(pc * P, (pc + 1) * P)
                    if h == 0:
                        nc.gpsimd.memset(xs[:, 0:1], 0.0)
                        nc.sync.dma_start(xs[:, 1:wcols],
                                          x_d[s, rows, 0:hw + 1])
                    elif h == lh_n - 1:
                        nc.gpsimd.memset(xs[:, wcols - 1:wcols], 0.0)
                        nc.sync.dma_start(xs[:, 0:wcols - 1],
                                          x_d[s, rows, h * hw - 1:length])
                    else:
                        nc.sync.dma_start(xs[:, :],
                                          x_d[s, rows,
                                              h * hw - 1:(h + 1) * hw + 1])
                    hi = hi_p.tile([P, wcols], f32r, tag="hi")
                    lo = lo_p.tile([P, wcols], f32r, tag="lo")
                    nc.vector.tensor_copy(hi[:], xs[:])
                    nc.vector.tensor_tensor(lo[:], xs[:], hi[:],
                                            op=Alu.subtract)
                    his.append(hi)
                    los.append(lo)

                for oc in range(oc_n):
                    ps = psum_p.tile([P, hw], f32, tag="psum")
                    j = 0
                    for src in (his, los):
                        for pc in range(pc_n):
                            for k in range(kw):
                                lhsT = wsgn[k * pc_n + pc][
                                    :, oc * P:(oc + 1) * P]
                                start = j == 0
                                stop = j == n_acc - 1
                                for lt in range(lt_n):
                                    nc.tensor.matmul(
                                        ps[:, lt * NTILE:(lt + 1) * NTILE],
                                        lhsT,
                                        src[pc][:, lt * NTILE + k:
                                                lt * NTILE + k + NTILE],
                                        start=start, stop=stop,
                                    )
                                j += 1
                    half_w = hw // 2
                    for q in range(2):
                        ot = out_p.tile([P, half_w], f32, tag="outs")
                        nc.scalar.activation(
                            ot[:], ps[:, q * half_w:(q + 1) * half_w],
                            Act.Copy, scale=cb[:],
                        )
                        nc.sync.dma_start(
                            o_d[s, oc * P:(oc + 1) * P,
                                h * hw + q * half_w:
                                h * hw + (q + 1) * half_w],
                            ot[:],
                        )

    nc.compile()
    return nc


def _get_nc(key=(NS, C, L, KW)):
    if key not in _CACHE:
        _CACHE[key] = _build_nc(*key)
    return _CACHE[key]


def _shard_inputs(x, weight, scale):
    x = np.ascontiguousarray(np.asarray(x, dtype=np.float32))
    weight = np.asarray(weight, dtype=np.float32)
    scale = np.asarray(scale, dtype=np.float32).reshape(1, 1)
    # [C_out, C_in, K] -> [K, C_in, C_out] so DMA reads are contiguous
    wt = np.ascontiguousarray(weight.transpose(2, 1, 0))
    return [
        {"x": x[i * NS:(i + 1) * NS], "wt": wt, "scale": scale}
        for i in range(NCORES)
    ]


def run_shards(in_maps, trace=False, **kw):
    from concourse.bass_utils import run_bass_kernel_spmd

    nc = _get_nc()
    return run_bass_kernel_spmd(nc, in_maps, list(range(NCORES)),
                                trace=trace, **kw)


def kernel(x, weight, scale):
    res = run_shards(_shard_inputs(x, weight, scale))
    return np.concatenate([r["out"] for r in res.results], axis=0)


# revision 11
# speedup vs baseline: 17.7788x; 17.7788x over previous
"""BitConv1d Trainium2 kernel.

Computes out[n,o,l] = conv1d(x, sign(w), pad=1) * mean(|w|) * scale, which is
mathematically identical to the reference

    x_scale = clip(mean(|x|, axis=(1,2)), 1e-5)
    out = conv1d(x / x_scale, sign(w), pad=1) * mean(|w|) * x_scale * scale

because conv is linear in x so the per-sample x_scale cancels exactly.

Sharding: data-parallel over batch N=16 across 8 cores (2 samples/core).

Device math: the PE array's native datapath is FP22 (e10m11).  float32
matmuls cost 4 passes; float32r costs 1 pass but rounds operands to
FP22.  Since sign(w) ∈ {-1,0,1} is FP22-exact, we split
    hi = round_fp22(x)    (DVE f32 -> f32r convert on write)
    lo = x - hi           (<= 12 significant bits -> FP22-exact)
and accumulate two float32r matmul passes into fp32 PSUM.  Both operands
are FP22-exact in SBUF, so every PE product is exact and the result
matches a true fp32 conv up to fp32 accumulation order, at 2x the
bf16-rate cost instead of 4x.
"""

import numpy as np

# Problem geometry (hardcoded per contract).
N, C, L, KW = 16, 512, 4096, 3
NCORES = 8
NS = N // NCORES          # samples per core
P = 128                   # partitions
PC = C // P               # input-channel chunks
OC = C // P               # output-channel chunks
NTILE = 512               # moving free-dim per matmul

_CACHE = {}


def _build_nc(ns=NS, c=C, length=L, kw=KW, repeat=1):
    from contextlib import ExitStack
    from concourse import bacc, tile, mybir

    f32 = mybir.dt.float32
    f32r = mybir.dt.float32r
    Alu = mybir.AluOpType
    Act = mybir.ActivationFunctionType

    pc_n = c // P
    oc_n = c // P
    lh_n = 2                  # L halves (psum double buffering)
    hw = length // lh_n       # columns per half
    lt_n = hw // NTILE        # matmul tiles per half
    wcols = hw + 2            # with 1-col halo on each side

    nc = bacc.Bacc("TRN2", target_bir_lowering=False, debug=False)

    x_d = nc.dram_tensor("x", [ns, c, length], f32, kind="ExternalInput")
    w_d = nc.dram_tensor("wt", [kw, c, c], f32, kind="ExternalInput")
    s_d = nc.dram_tensor("scale", [1, 1], f32, kind="ExternalInput")
    o_d = nc.dram_tensor("out", [ns, c, length], f32, kind="ExternalOutput")

    with tile.TileContext(nc) as tc, ExitStack() as ctx:
        consts = ctx.enter_context(tc.tile_pool(name="consts", bufs=1))
        wst_p = ctx.enter_context(tc.tile_pool(name="wst", bufs=2))
        wsgn_p = ctx.enter_context(tc.tile_pool(name="wsgn", bufs=kw * pc_n))
        xs_p = ctx.enter_context(tc.tile_pool(name="xs", bufs=2))
        hi_p = ctx.enter_context(tc.tile_pool(name="hi", bufs=2 * pc_n))
        lo_p = ctx.enter_context(tc.tile_pool(name="lo", bufs=2 * pc_n))
        out_p = ctx.enter_context(tc.tile_pool(name="outs", bufs=2))
        psum_p = ctx.enter_context(
            tc.tile_pool(name="psum", bufs=2, space="PSUM")
        )

        # ---------- setup: scale, sign(w), w_scale ----------
        sc = consts.tile([1, 1], f32, tag="sc")
        nc.sync.dma_start(sc[:, :], s_d[:, :])

        ones_col = consts.tile([P, 1], f32, tag="ones_col")
        nc.gpsimd.memset(ones_col[:], 1.0)
        ones_row = consts.tile([1, P], f32, tag="ones_row")
        nc.gpsimd.memset(ones_row[:], 1.0)

        partials = consts.tile([P, kw * pc_n], f32, tag="partials")
        wsgn = []
        for k in range(kw):
            for pc in range(pc_n):
                wst = wst_p.tile([P, c], f32, tag="wst")
                nc.sync.dma_start(wst[:], w_d[k, pc * P:(pc + 1) * P, :])
                j = k * pc_n + pc
                nc.vector.tensor_reduce(
                    partials[:, j:j + 1], wst[:], mybir.AxisListType.X,
                    Alu.add, apply_absolute_value=True,
                )
                wt = wsgn_p.tile([P, c], f32r, tag="wsgn")
                nc.scalar.sign(wt[:], wst[:])
                wsgn.append(wt)

        part1 = consts.tile([P, 1], f32, tag="part1")
        nc.vector.tensor_reduce(
            part1[:], partials[:], mybir.AxisListType.X, Alu.add
        )
        ps0 = psum_p.tile([P, hw], f32, tag="psum")
        nc.tensor.matmul(ps0[0:1, 0:1], part1[:], ones_col[:],
                         start=True, stop=True)
        tot = consts.tile([1, 1], f32, tag="tot")
        nc.vector.tensor_copy(tot[:], ps0[0:1, 0:1])
        c1 = consts.tile([1, 1], f32, tag="c1")
        nc.vector.tensor_tensor(c1[:], tot[:], sc[:], op=Alu.mult)
        nc.vector.tensor_scalar_mul(c1[:], c1[:], 1.0 / (c * c * kw))
        ps1 = psum_p.tile([P, hw], f32, tag="psum")
        nc.tensor.matmul(ps1[:, 0:1], ones_row[:], c1[:],
                         start=True, stop=True)
        cb = consts.tile([P, 1], f32, tag="cb")
        nc.vector.tensor_copy(cb[:], ps1[:, 0:1])

        # ---------- main loop ----------
        n_acc = 2 * pc_n * kw
        for s in [si for _ in range(repeat) for si in range(ns)]:
            for h in range(lh_n):
                his, los = [], []
                for pc in range(pc_n):
                    xs = xs_p.tile([P, wcols], f32, tag="xs")
                    rows = slice(pc * P, (pc + 1) * P)
                    if h == 0:
                        nc.gpsimd.memset(xs[:, 0:1], 0.0)
                        nc.sync.dma_start(xs[:, 1:wcols],
                                          x_d[s, rows, 0:hw + 1])
                    elif h == lh_n - 1:
                        nc.gpsimd.memset(xs[:, wcols - 1:wcols], 0.0)
                        nc.sync.dma_start(xs[:, 0:wcols - 1],
                                          x_d[s, rows, h * hw - 1:length])
                    else:
                        nc.sync.dma_start(xs[:, :],
                                          x_d[s, rows,
                                              h * hw - 1:(h + 1) * hw + 1])
                    hi = hi_p.tile([P, wcols], f32r, tag="hi")
                    lo = lo_p.tile([P, wcols], f32r, tag="lo")
                    nc.vector.tensor_copy(hi[:], xs[:])
                    nc.vector.tensor_tensor(lo[:], xs[:], hi[:],
                                            op=Alu.subtract)
                    his.append(hi)
                    los.append(lo)

                for oc in range(oc_n):
                    ps = psum_p.tile([P, hw], f32, tag="psum")
                    j = 0
                    for src in (his, los):
                        for pc in range(pc_n):
                            for k in range(kw):
                                lhsT = wsgn[k * pc_n + pc][
                                    :, oc * P:(oc + 1) * P]
                                start = j == 0
                                stop = j == n_acc - 1
                                for lt in range(lt_n):
                                    nc.tensor.matmul(
                                        ps[:, lt * NTILE:(lt + 1) * NTILE],
                                        lhsT,
                                        src[pc][:, lt * NTILE + k:
                                                lt * NTILE + k + NTILE],
                                        start=start, stop=stop,
                                    )
                                j += 1
                    half_w = hw // 2
                    for q in range(2):
                        ot = out_p.tile([P, half_w], f32, tag="outs")
                        nc.scalar.activation(
                            ot[:], ps[:, q * half_w:(q + 1) * half_w],
                            Act.Copy, scale=cb[:],
                        )
                        nc.sync.dma_start(
                            o_d[s, oc * P:(oc + 1) * P,
                                h * hw + q * half_w:
                                h * hw + (q + 1) * half_w],
                            ot[:],
                        )

    nc.compile()
    return nc


def _get_nc(key=(NS, C, L, KW)):
    if key not in _CACHE:
        _CACHE[key] = _build_nc(*key)
    return _CACHE[key]


def _shard_inputs(x, weight, scale):
    x = np.ascontiguousarray(np.asarray(x, dtype=np.float32))
    weight = np.asarray(weight, dtype=np.float32)
    scale = np.asarray(scale, dtype=np.float32).reshape(1, 1)
    # [C_out, C_in, K] -> [K, C_in, C_out] so DMA reads are contiguous
    wt = np.ascontiguousarray(weight.transpose(2, 1, 0))
    return [
        {"x": x[i * NS:(i + 1) * NS], "wt": wt, "scale": scale}
        for i in range(NCORES)
    ]


def run_shards(in_maps, trace=False, **kw):
    from concourse.bass_utils import run_bass_kernel_spmd

    nc = _get_nc()
    return run_bass_kernel_spmd(nc, in_maps, list(range(NCORES)),
                                trace=trace, **kw)


def kernel(x, weight, scale):
    res = run_shards(_shard_inputs(x, weight, scale))
    return np.concatenate([r["out"] for r in res.results], axis=0)
